# revision 3
# baseline (speedup 1.0000x reference)
"""AceStep cross-attention on 8 trn2 NeuronCores.

Sharding: each core owns one (batch, query-half) pair — batch b = core//2,
query rows [half*1024, (half+1)*1024) of S=2048. Cross-attention rows are
independent across query positions, so each core computes its full output
slice with no inter-core reduction; k/v projections for a batch are
(cheaply) duplicated across its two cores.

Per-core dataflow (all matmuls bf16 with fp32 PSUM accumulation):
  A: k = enc @ Wk, v = enc @ Wv (natural layout), RMS-norm k per head,
     PE-transpose k -> kT [d, s_enc] with (q_norm_w*k_norm_w) folded in.
  B: q = hs @ Wq (natural), RMS-norm q, PE-transpose -> qT [d, q].
  C: per head: scoresT[k,q] = kT.T@qT; probsT = exp(scale*scoresT) (no max
     subtraction needed: RMS-normed q,k give O(1) scores, fp32 exp is safe);
     denom = ones.T @ probsT (PE); outT[d,q] = v.T-matmul over k blocks;
     attnT = outT * (1/denom) broadcast.
  D: out[q, hidden] = attnT.T @ Wo, fp32 out.
"""

import math

import numpy as np
import ml_dtypes

HIDDEN = 2048
NH = 16
NKV = 4
D = 128
B = 4
S = 2048
SENC = 1024
SLOC = S // 2  # query rows per core
EPS = 1e-6
P = 128
HC = HIDDEN // P  # hidden chunks
QB = SLOC // P    # query blocks
SB = SENC // P    # encoder seq blocks
GROUPS = NH // NKV

_CACHE = {}


def _build():
    from contextlib import ExitStack

    import concourse.bass as bass
    import concourse.mybir as mybir
    import concourse.tile as tile
    from concourse import bacc
    from concourse.masks import make_identity

    dt = mybir.dt
    ts = bass.ts
    AF = mybir.ActivationFunctionType
    ALU = mybir.AluOpType

    nc = bacc.Bacc()

    hsT = nc.dram_tensor("hsT", [HIDDEN, SLOC], dt.bfloat16, kind="ExternalInput")
    encT = nc.dram_tensor("encT", [HIDDEN, SENC], dt.bfloat16, kind="ExternalInput")
    wq = nc.dram_tensor("wq", [HIDDEN, NH * D], dt.bfloat16, kind="ExternalInput")
    wk = nc.dram_tensor("wk", [HIDDEN, NKV * D], dt.bfloat16, kind="ExternalInput")
    wv = nc.dram_tensor("wv", [HIDDEN, NKV * D], dt.bfloat16, kind="ExternalInput")
    wo = nc.dram_tensor("wo", [NH * D, HIDDEN], dt.bfloat16, kind="ExternalInput")
    wqk = nc.dram_tensor("wqk", [D, 1], dt.float32, kind="ExternalInput")
    outd = nc.dram_tensor("out", [SLOC, HIDDEN], dt.float32, kind="ExternalOutput")

    hsr = hsT.rearrange("(hc p) s -> p hc s", p=P)
    encr = encT.rearrange("(hc p) s -> p hc s", p=P)
    wqr = wq.rearrange("(hc p) n -> p hc n", p=P)
    wkr = wk.rearrange("(hc p) n -> p hc n", p=P)
    wvr = wv.rearrange("(hc p) n -> p hc n", p=P)
    wor = wo.rearrange("(h d) n -> d h n", d=P)

    scale = 1.0 / math.sqrt(D)

    with tile.TileContext(nc) as tc, ExitStack() as ctx:
        persist = ctx.enter_context(tc.tile_pool(name="persist", bufs=1))
        kT = persist.tile([P, NKV, SENC], dt.bfloat16)
        vsb = persist.tile([P, SB, NKV * D], dt.bfloat16)
        qT = persist.tile([P, NH, SLOC], dt.bfloat16)
        attnT = persist.tile([P, NH, SLOC], dt.bfloat16)
        ident = persist.tile([P, P], dt.bfloat16)
        make_identity(nc, ident)
        ones = persist.tile([P, 1], dt.bfloat16)
        nc.vector.memset(ones, 1.0)
        wqk_sb = persist.tile([P, 1], dt.float32)
        nc.sync.dma_start(out=wqk_sb, in_=wqk[:, :])
        eps_sb = persist.tile([P, 1], dt.float32)
        nc.vector.memset(eps_sb, EPS)

        stats = ctx.enter_context(tc.tile_pool(name="stats", bufs=3))

        # hs is loaded early (own pool, freed after stage B) so its DMA
        # overlaps stage A compute.
        hs_ctx = tc.tile_pool(name="hs_pool", bufs=1)
        hs_pool = hs_ctx.__enter__()
        hs_sb = hs_pool.tile([P, HC, SLOC], dt.bfloat16)
        for hc in range(HC):
            nc.sync.dma_start(out=hs_sb[:, hc, :], in_=hsr[:, hc, :])

        # ---------------- Stage A: k/v projections + k norm/transpose -----
        with tc.tile_pool(name="stageA", bufs=1) as pa, \
             tc.tile_pool(name="psA", bufs=2, space="PSUM") as psA:
            enc_sb = pa.tile([P, HC, SENC], dt.bfloat16)
            wk_sb = pa.tile([P, HC, NKV * D], dt.bfloat16)
            wv_sb = pa.tile([P, HC, NKV * D], dt.bfloat16)
            for hc in range(HC):
                nc.sync.dma_start(out=wk_sb[:, hc, :], in_=wkr[:, hc, :])
                nc.sync.dma_start(out=wv_sb[:, hc, :], in_=wvr[:, hc, :])
                nc.sync.dma_start(out=enc_sb[:, hc, :], in_=encr[:, hc, :])

            for sb_i in range(SB):
                kp = psA.tile([P, NKV * D], dt.float32, tag="kp")
                vp = psA.tile([P, NKV * D], dt.float32, tag="vp")
                for hc in range(HC):
                    nc.tensor.matmul(
                        kp[:], enc_sb[:, hc, ts(sb_i, P)], wk_sb[:, hc, :],
                        start=hc == 0, stop=hc == HC - 1)
                for hc in range(HC):
                    nc.tensor.matmul(
                        vp[:], enc_sb[:, hc, ts(sb_i, P)], wv_sb[:, hc, :],
                        start=hc == 0, stop=hc == HC - 1)
                nc.vector.tensor_copy(vsb[:, sb_i, :], vp[:])

                sumsq = stats.tile([P, NKV], dt.float32, tag="sumsq")
                for g in range(NKV):
                    sq = stats.tile([P, D], dt.float32, tag="sq")
                    nc.scalar.activation(
                        out=sq[:], in_=kp[:, ts(g, D)], func=AF.Square,
                        accum_out=sumsq[:, g:g + 1])
                std = stats.tile([P, NKV], dt.float32, tag="std")
                nc.scalar.activation(
                    out=std[:], in_=sumsq[:], func=AF.Sqrt,
                    bias=eps_sb[:], scale=1.0 / D)
                rstd = stats.tile([P, NKV], dt.float32, tag="rstd")
                nc.vector.reciprocal(rstd[:], std[:])

                kn = stats.tile([P, NKV, D], dt.bfloat16, tag="kn")
                nc.vector.tensor_tensor(
                    out=kn[:],
                    in0=kp[:].rearrange("p (g d) -> p g d", g=NKV),
                    in1=rstd[:, :, None].broadcast_to([P, NKV, D]),
                    op=ALU.mult)
                for g in range(NKV):
                    tp = psA.tile([P, P], dt.bfloat16, tag="tp")
                    nc.tensor.transpose(tp[:], kn[:, g, :], ident[:])
                    nc.vector.tensor_scalar(
                        out=kT[:, g, ts(sb_i, P)], in0=tp[:],
                        scalar1=wqk_sb[:], scalar2=None, op0=ALU.mult)

        # ---------------- Stage B: q projection + norm/transpose ----------
        with tc.tile_pool(name="stageB", bufs=1) as pb, \
             tc.tile_pool(name="psB", bufs=2, space="PSUM") as psB:
            wq_sb = pb.tile([P, HC, NH * D], dt.bfloat16)
            for hc in range(HC):
                nc.sync.dma_start(out=wq_sb[:, hc, :], in_=wqr[:, hc, :])

            NIC = (NH * D) // 512  # inner chunks of 512 (4 heads each)
            for qb in range(QB):
                for ic in range(NIC):
                    qp = psB.tile([P, 512], dt.float32, tag="qp")
                    for hc in range(HC):
                        nc.tensor.matmul(
                            qp[:], hs_sb[:, hc, ts(qb, P)],
                            wq_sb[:, hc, ts(ic, 512)],
                            start=hc == 0, stop=hc == HC - 1)
                    sumsq = stats.tile([P, 4], dt.float32, tag="sumsq")
                    for j in range(4):
                        sq = stats.tile([P, D], dt.float32, tag="sq")
                        nc.scalar.activation(
                            out=sq[:], in_=qp[:, ts(j, D)], func=AF.Square,
                            accum_out=sumsq[:, j:j + 1])
                    std = stats.tile([P, 4], dt.float32, tag="std")
                    nc.scalar.activation(
                        out=std[:], in_=sumsq[:], func=AF.Sqrt,
                        bias=eps_sb[:], scale=1.0 / D)
                    rstd = stats.tile([P, 4], dt.float32, tag="rstd")
                    nc.vector.reciprocal(rstd[:], std[:])

                    qn = stats.tile([P, 4, D], dt.bfloat16, tag="qn")
                    nc.vector.tensor_tensor(
                        out=qn[:],
                        in0=qp[:].rearrange("p (g d) -> p g d", g=4),
                        in1=rstd[:, :, None].broadcast_to([P, 4, D]),
                        op=ALU.mult)
                    for j in range(4):
                        h = ic * 4 + j
                        tp = psB.tile([P, P], dt.bfloat16, tag="tp")
                        nc.tensor.transpose(tp[:], qn[:, j, :], ident[:])
                        nc.vector.tensor_copy(qT[:, h, ts(qb, P)], tp[:])

        hs_ctx.__exit__(None, None, None)

        # ---------------- Stage C: attention ------------------------------
        NQC = SLOC // 512  # query chunks of 512
        with tc.tile_pool(name="probs_pool", bufs=2) as ppool, \
             tc.tile_pool(name="wo_pool", bufs=1) as pwo, \
             tc.tile_pool(name="psC", bufs=2, space="PSUM") as psC:
            wo_sb = pwo.tile([P, NH, HIDDEN], dt.bfloat16)
            for h in range(NH):
                nc.sync.dma_start(out=wo_sb[:, h, :], in_=wor[:, h, :])

            for h in range(NH):
                g = h // GROUPS
                probsT = ppool.tile([P, SB, SLOC], dt.bfloat16, tag="probsT")
                for qc in range(NQC):
                    for kb in range(SB):
                        sp = psC.tile([P, 512], dt.float32, tag="sp")
                        nc.tensor.matmul(
                            sp[:], kT[:, g, ts(kb, P)], qT[:, h, ts(qc, 512)],
                            start=True, stop=True)
                        nc.scalar.activation(
                            out=probsT[:, kb, ts(qc, 512)], in_=sp[:],
                            func=AF.Exp, scale=scale)
                    dn = psC.tile([1, 512], dt.float32, tag="dn")
                    for kb in range(SB):
                        nc.tensor.matmul(
                            dn[:], ones[:], probsT[:, kb, ts(qc, 512)],
                            start=kb == 0, stop=kb == SB - 1)
                    op = psC.tile([P, 512], dt.float32, tag="op")
                    for kb in range(SB):
                        nc.tensor.matmul(
                            op[:], vsb[:, kb, ts(g, D)],
                            probsT[:, kb, ts(qc, 512)],
                            start=kb == 0, stop=kb == SB - 1)
                    rdn = stats.tile([1, 512], dt.float32, tag="rdn")
                    nc.vector.reciprocal(rdn[:], dn[:])
                    rdnb = stats.tile([P, 512], dt.float32, tag="rdnb")
                    nc.gpsimd.partition_broadcast(rdnb[:], rdn[:])
                    nc.vector.tensor_tensor(
                        out=attnT[:, h, ts(qc, 512)], in0=op[:], in1=rdnb[:],
                        op=ALU.mult)

            # ------------- Stage D: output projection ---------------------
            NOC = HIDDEN // 512
            with tc.tile_pool(name="psD", bufs=2, space="PSUM") as psD:
                for qb in range(QB):
                    for oc in range(NOC):
                        outp = psD.tile([P, 512], dt.float32, tag="outp")
                        for h in range(NH):
                            nc.tensor.matmul(
                                outp[:], attnT[:, h, ts(qb, P)],
                                wo_sb[:, h, ts(oc, 512)],
                                start=h == 0, stop=h == NH - 1)
                        ot = stats.tile([P, 512], dt.float32, tag="ot")
                        nc.vector.tensor_copy(ot[:], outp[:])
                        nc.sync.dma_start(
                            out=outd[ts(qb, P), ts(oc, 512)], in_=ot[:])

    nc.finalize()
    return nc


def kernel(hidden_states, encoder_hidden_states, Wq, Wk, Wv, Wo,
           q_norm_w, k_norm_w, _trace=False):
    from concourse.bass_utils import run_bass_kernel_spmd

    bf16 = ml_dtypes.bfloat16
    wq_b = np.ascontiguousarray(Wq).astype(bf16)
    wk_b = np.ascontiguousarray(Wk).astype(bf16)
    wv_b = np.ascontiguousarray(Wv).astype(bf16)
    wo_b = np.ascontiguousarray(Wo).astype(bf16)
    wqk = (np.asarray(q_norm_w, np.float32)
           * np.asarray(k_norm_w, np.float32)).reshape(D, 1)

    encT_b = [np.ascontiguousarray(encoder_hidden_states[b].T).astype(bf16)
              for b in range(B)]

    in_maps = []
    for c in range(8):
        b, half = divmod(c, 2)
        hsT_b = np.ascontiguousarray(
            hidden_states[b, half * SLOC:(half + 1) * SLOC, :].T).astype(bf16)
        in_maps.append({
            "hsT": hsT_b, "encT": encT_b[b],
            "wq": wq_b, "wk": wk_b, "wv": wv_b, "wo": wo_b,
            "wqk": wqk,
        })

    if "nc" not in _CACHE:
        _CACHE["nc"] = _build()
    nc = _CACHE["nc"]

    res = run_bass_kernel_spmd(nc, in_maps, core_ids=list(range(8)),
                               trace=_trace)
    out = np.empty((B, S, HIDDEN), np.float32)
    for c in range(8):
        b, half = divmod(c, 2)
        out[b, half * SLOC:(half + 1) * SLOC, :] = res.results[c]["out"]
    if _trace:
        _CACHE["last_result"] = res
    return out
